# revision 11
# baseline (speedup 1.0000x reference)
"""Trainium2 Bass kernel for Lorentz batch-norm (hyperboloid model).

Data-parallel over N across 8 NeuronCores. Per core:
  pass A: stream x, column-sum -> AllReduce -> Lorentz centroid mu
  pass B: stream x, per-row alpha = rowdot(x, J mu), d = arccosh(alpha),
          partial sum(d^2) -> AllReduce -> var -> div
  per-row coefficient pipeline: the whole transform collapses to
          out_row = A*x_row + B*mu + C*e0 + D*beta  (per-row scalars A..D)
  pass C: stream x, combine with broadcast APs, write out.

kernel(**inputs) takes FULL inputs (numpy), shards rows 8 ways, runs the
SPMD program via run_bass_kernel_spmd, concatenates the per-core outputs.
"""

import os
import sys

import numpy as np

for _p in ("/opt/trn_rl_repo", "/root/.axon_site/_ro/trn_rl_repo"):
    if os.path.isdir(_p) and _p not in sys.path:
        sys.path.insert(0, _p)

import concourse.bass as bass
import concourse.bacc as bacc
import concourse.tile as tile
from concourse import mybir
from concourse.bass_utils import run_bass_kernel_spmd

FP = mybir.dt.float32
N_CORES = 8
N_TOTAL = 1048576
C = 64
P = 128                      # SBUF partitions
K = 32                       # rows per partition per tile
EPS = 1e-5
ACOSH_EPS = 1e-7

AX = mybir.AxisListType
OP = mybir.AluOpType
AF = mybir.ActivationFunctionType


def build_program(rows_per_core: int, n_total: int, beta_origin: bool) -> bass.Bass:
    R = rows_per_core
    assert R % (P * K) == 0
    T = R // (P * K)          # number of tiles
    F = K * C                 # tile free size
    RPP = R // P              # per-row array width (= K*T)

    nc = bacc.Bacc(num_devices=N_CORES)

    x_in = nc.declare_dram_parameter("x", [R, C], FP, isOutput=False)
    g_in = nc.declare_dram_parameter("gamma", [1], FP, isOutput=False)
    b_in = nc.declare_dram_parameter("beta", [C], FP, isOutput=False)
    out_d = nc.declare_dram_parameter("out", [R, C], FP, isOutput=True)

    x_view = x_in.ap().rearrange("(t p k) c -> t p (k c)", p=P, k=K)
    o_view = out_d.ap().rearrange("(t p k) c -> t p (k c)", p=P, k=K)

    with tile.TileContext(nc) as tc:
        from contextlib import ExitStack

        with ExitStack() as ctx:
            const = ctx.enter_context(tc.tile_pool(name="const", bufs=1))
            xpool = ctx.enter_context(tc.tile_pool(name="xp", bufs=4))
            opool = ctx.enter_context(tc.tile_pool(name="op", bufs=3))
            wpool = ctx.enter_context(tc.tile_pool(name="wp", bufs=2))
            rows = ctx.enter_context(tc.tile_pool(name="rows", bufs=1))
            tmp = ctx.enter_context(tc.tile_pool(name="tmp", bufs=1))
            psum = ctx.enter_context(
                tc.tile_pool(name="ps", bufs=1, space="PSUM")
            )
            dram = ctx.enter_context(tc.tile_pool(name="dr", bufs=1, space="DRAM"))

            v = nc.vector
            sc = nc.scalar

            # ---------------- constants -----------------------------------
            ones_col = const.tile([1, P], FP, tag="ones_col")   # K=1 bcast lhsT
            v.memset(ones_col[:], 1.0)
            ones128 = const.tile([P, 1], FP, tag="ones128")     # col-sum lhsT
            v.memset(ones128[:], 1.0)
            jsign = const.tile([1, C], FP, tag="jsign")         # (1,-1,...,-1)
            v.memset(jsign[:], -1.0)
            v.memset(jsign[0:1, 0:1], 1.0)

            beta_row = const.tile([1, C], FP, tag="beta_row")
            nc.gpsimd.dma_start(out=beta_row[:], in_=b_in.ap().unsqueeze(0))
            gamma_sb = const.tile([1, 1], FP, tag="gamma_sb")
            nc.gpsimd.dma_start(out=gamma_sb[:], in_=g_in.ap().unsqueeze(0))
            jbeta_row = const.tile([1, C], FP, tag="jbeta_row")
            v.tensor_mul(jbeta_row[:], beta_row[:], jsign[:])

            # ---------------- pass A: column sums -------------------------
            acc = rows.tile([P, F], FP, tag="acc")
            for t in range(T):
                x_t = xpool.tile([P, F], FP, tag="x_t")
                nc.sync.dma_start(out=x_t[:], in_=x_view[t])
                if t == 0:
                    v.tensor_copy(acc[:], x_t[:])
                else:
                    v.tensor_add(acc[:], acc[:], x_t[:])

            # fold K rows-per-partition: view [P, K, C] -> transpose -> [P, C, K]
            acc_t = acc[:].rearrange("p (k c) -> p k c", c=C).transpose([0, 2, 1])
            s_p = tmp.tile([P, C], FP, tag="s_p")
            v.tensor_reduce(s_p[:], acc_t, axis=AX.X, op=OP.add)

            ps_s = psum.tile([1, C], FP, tag="ps_s")
            nc.tensor.matmul(ps_s[:], ones128[:], s_p[:], start=True, stop=True)
            s_row = const.tile([1, C], FP, tag="s_row")
            sc.copy(s_row[:], ps_s[:])

            # AllReduce s
            cc_in = dram.tile([1, C], FP, tag="cc_in")
            cc_out = dram.tile([1, C], FP, tag="cc_out")
            nc.gpsimd.dma_start(out=cc_in[:], in_=s_row[:])
            nc.gpsimd.collective_compute(
                "AllReduce",
                OP.add,
                replica_groups=[list(range(N_CORES))],
                ins=[cc_in.opt()],
                outs=[cc_out.opt()],
            )
            sg_row = const.tile([1, C], FP, tag="sg_row")
            nc.gpsimd.dma_start(out=sg_row[:], in_=cc_out[:])

            # ---------------- mu on partition 0 ---------------------------
            def nr_rsqrt(y, z, steps=2):
                """Refine y ~= 1/sqrt(z) in place (per-element tiles)."""
                t1 = tmp.tile(list(y.shape), FP, tag="nr_t1", name="nr_t1")
                for _ in range(steps):
                    v.tensor_mul(t1[:], y[:], y[:])
                    v.tensor_mul(t1[:], t1[:], z[:])
                    v.tensor_scalar(t1[:], t1[:], -0.5, 1.5, OP.mult, OP.add)
                    v.tensor_mul(y[:], y[:], t1[:])

            sq64 = tmp.tile([1, C], FP, tag="sq64")
            sc.activation(sq64[:], sg_row[:], AF.Square)
            ssum = tmp.tile([1, 1], FP, tag="ssum")
            v.tensor_reduce(ssum[:], sq64[:], axis=AX.X, op=OP.add)
            snsq = tmp.tile([1, 1], FP, tag="snsq")
            # snsq = -<s,s>_L = 2*s0^2 - sum(s^2)
            v.tensor_scalar(snsq[:], sq64[0:1, 0:1], 2.0, None, OP.mult)
            v.tensor_sub(snsq[:], snsq[:], ssum[:])
            v.tensor_scalar(snsq[:], snsq[:], 1e-9, None, OP.max)
            rsn = tmp.tile([1, 1], FP, tag="rsn")
            rtmp = tmp.tile([1, 1], FP, tag="rtmp")
            v.reciprocal(rtmp[:], snsq[:])
            sc.activation(rsn[:], rtmp[:], AF.Sqrt)
            nr_rsqrt(rsn, snsq)

            mu_row = const.tile([1, C], FP, tag="mu_row")
            v.tensor_scalar(mu_row[:], sg_row[:], rsn[0:1, 0:1], None, OP.mult)
            jmu_row = const.tile([1, C], FP, tag="jmu_row")
            v.tensor_mul(jmu_row[:], mu_row[:], jsign[:])

            # broadcast mu, Jmu to all partitions via PE
            ps_b = psum.tile([P, C], FP, tag="ps_b")
            mu_t = const.tile([P, C], FP, tag="mu_t")
            nc.tensor.matmul(ps_b[:], ones_col[:], mu_row[:], start=True, stop=True)
            sc.copy(mu_t[:], ps_b[:])
            ps_b2 = psum.tile([P, C], FP, tag="ps_b2")
            jmu_t = const.tile([P, C], FP, tag="jmu_t")
            nc.tensor.matmul(ps_b2[:], ones_col[:], jmu_row[:], start=True, stop=True)
            sc.copy(jmu_t[:], ps_b2[:])

            if not beta_origin:
                jbeta_t = const.tile([P, C], FP, tag="jbeta_t")
                ps_b3 = psum.tile([P, C], FP, tag="ps_b3")
                nc.tensor.matmul(
                    ps_b3[:], ones_col[:], jbeta_row[:], start=True, stop=True
                )
                sc.copy(jbeta_t[:], ps_b3[:])
                beta_t = const.tile([P, C], FP, tag="beta_t")
                ps_b4 = psum.tile([P, C], FP, tag="ps_b4")
                nc.tensor.matmul(
                    ps_b4[:], ones_col[:], beta_row[:], start=True, stop=True
                )
                sc.copy(beta_t[:], ps_b4[:])

            # ---------------- pass B: alpha, x0 (and xb) ------------------
            alpha_all = rows.tile([P, RPP], FP, tag="alpha_all")
            x0_all = rows.tile([P, RPP], FP, tag="x0_all")
            if not beta_origin:
                xb_all = rows.tile([P, RPP], FP, tag="xb_all")
            jmu_bc = jmu_t[:].unsqueeze(1).broadcast_to([P, K, C])
            if not beta_origin:
                jbeta_bc = jbeta_t[:].unsqueeze(1).broadcast_to([P, K, C])

            for t in range(T):
                x_t = xpool.tile([P, F], FP, tag="x_t")
                nc.sync.dma_start(out=x_t[:], in_=x_view[t])
                x3 = x_t[:].rearrange("p (k c) -> p k c", c=C)
                xm = wpool.tile([P, F], FP, tag="xm")
                xm3 = xm[:].rearrange("p (k c) -> p k c", c=C)
                v.tensor_mul(xm3, x3, jmu_bc)
                v.tensor_reduce(
                    alpha_all[:, bass.ts(t, K)], xm3, axis=AX.X, op=OP.add
                )
                v.tensor_copy(x0_all[:, bass.ts(t, K)], x3[:, :, 0:1].squeeze(2))
                if not beta_origin:
                    xm2 = wpool.tile([P, F], FP, tag="xm2")
                    xm23 = xm2[:].rearrange("p (k c) -> p k c", c=C)
                    v.tensor_mul(xm23, x3, jbeta_bc)
                    v.tensor_reduce(
                        xb_all[:, bass.ts(t, K)], xm23, axis=AX.X, op=OP.add
                    )

            xb_arr = x0_all if beta_origin else xb_all

            # ---------------- per-row pipeline part 1: d, f, var ----------
            def rt(tag):
                return rows.tile([P, RPP], FP, tag=tag, name=tag)

            # scratch slots, reused aggressively (bufs=1, WAR deps serialize)
            def st(i):
                return rows.tile([P, RPP], FP, tag=f"s{i}", name=f"s{i}")

            z = st(0)
            # clip alpha
            v.tensor_scalar(alpha_all[:], alpha_all[:], 1.0 + ACOSH_EPS, None, OP.max)
            sc.activation(z[:], alpha_all[:], AF.Square)
            v.tensor_scalar(z[:], z[:], -1.0, ACOSH_EPS, OP.add, OP.max)
            rz = st(1)
            rsq = st(2)
            v.reciprocal(rz[:], z[:])
            sc.activation(rsq[:], rz[:], AF.Sqrt)
            nr_rsqrt(rsq, z)
            sqz = st(1)
            v.tensor_mul(sqz[:], z[:], rsq[:])
            w = st(3)
            v.tensor_add(w[:], alpha_all[:], sqz[:])
            d = st(0)
            sc.activation(d[:], w[:], AF.Ln)
            f_all = rt("f_all")
            v.tensor_mul(f_all[:], d[:], rsq[:])

            vacc = tmp.tile([P, 1], FP, tag="vacc")
            dsq_scr = st(1)
            v.tensor_mul(dsq_scr[:], d[:], d[:])
            v.tensor_reduce(vacc[:], dsq_scr[:], axis=AX.X, op=OP.add)
            ps_v = psum.tile([1, 1], FP, tag="ps_v")
            nc.tensor.matmul(ps_v[:], ones128[:], vacc[:], start=True, stop=True)
            vs_row = tmp.tile([1, 1], FP, tag="vs_row")
            sc.copy(vs_row[:], ps_v[:])

            vcc_in = dram.tile([1, 1], FP, tag="vcc_in")
            vcc_out = dram.tile([1, 1], FP, tag="vcc_out")
            nc.gpsimd.dma_start(out=vcc_in[:], in_=vs_row[:])
            nc.gpsimd.collective_compute(
                "AllReduce",
                OP.add,
                replica_groups=[list(range(N_CORES))],
                ins=[vcc_in.opt()],
                outs=[vcc_out.opt()],
            )
            vsum = tmp.tile([1, 1], FP, tag="vsum")
            nc.gpsimd.dma_start(out=vsum[:], in_=vcc_out[:])

            # div = sqrt(var + EPS); G = gamma / div
            dv = tmp.tile([1, 1], FP, tag="dv")
            v.tensor_scalar(dv[:], vsum[:], 1.0 / float(n_total), EPS, OP.mult, OP.add)
            rdd = tmp.tile([1, 1], FP, tag="rdd")
            v.reciprocal(rdd[:], dv[:])
            rdiv = tmp.tile([1, 1], FP, tag="rdiv")
            sc.activation(rdiv[:], rdd[:], AF.Sqrt)
            nr_rsqrt(rdiv, dv)

            # ---------------- global scalar row + broadcast ---------------
            NSC = 16
            scrow = const.tile([1, NSC], FP, tag="scrow")
            mu0 = mu_row[0:1, 0:1]
            b0 = beta_row[0:1, 0:1]
            # 0: -mu0
            v.tensor_scalar(scrow[0:1, 0:1], mu0, -1.0, None, OP.mult)
            # 1: -1/(1+mu0)
            t11 = tmp.tile([1, 1], FP, tag="t11")
            v.tensor_scalar(t11[:], mu0, 1.0, None, OP.add)
            t12 = tmp.tile([1, 1], FP, tag="t12")
            v.reciprocal(t12[:], t11[:])
            v.tensor_scalar(scrow[0:1, 1:2], t12[:], -1.0, None, OP.mult)
            # 2: G = gamma * rdiv
            v.tensor_mul(scrow[0:1, 2:3], gamma_sb[:], rdiv[:])
            # 3: bmu = <beta,mu>_L = -sum(beta*Jmu)
            t64 = tmp.tile([1, C], FP, tag="t64")
            v.tensor_mul(t64[:], beta_row[:], jmu_row[:])
            t13 = tmp.tile([1, 1], FP, tag="t13")
            v.tensor_reduce(t13[:], t64[:], axis=AX.X, op=OP.add)
            v.tensor_scalar(scrow[0:1, 3:4], t13[:], -1.0, None, OP.mult)
            # 4: bo = -beta0
            v.tensor_scalar(scrow[0:1, 4:5], b0, -1.0, None, OP.mult)
            # 5: rdob = 1/(1+beta0)
            v.tensor_scalar(t11[:], b0, 1.0, None, OP.add)
            v.reciprocal(scrow[0:1, 5:6], t11[:])
            # 6: muu = <mu,mu>_L = -sum(mu*Jmu)
            v.tensor_mul(t64[:], mu_row[:], jmu_row[:])
            v.tensor_reduce(t13[:], t64[:], axis=AX.X, op=OP.add)
            v.tensor_scalar(scrow[0:1, 6:7], t13[:], -1.0, None, OP.mult)
            # 7: bb = <beta,beta>_L = -sum(beta*Jbeta)
            v.tensor_mul(t64[:], beta_row[:], jbeta_row[:])
            v.tensor_reduce(t13[:], t64[:], axis=AX.X, op=OP.add)
            v.tensor_scalar(scrow[0:1, 7:8], t13[:], -1.0, None, OP.mult)
            # 8: tmo = -2*mu0
            v.tensor_scalar(scrow[0:1, 8:9], mu0, -2.0, None, OP.mult)
            # 9: tmb = 2*bmu
            v.tensor_scalar(scrow[0:1, 9:10], scrow[0:1, 3:4], 2.0, None, OP.mult)
            # 10: -2*beta0
            v.tensor_scalar(scrow[0:1, 10:11], b0, -2.0, None, OP.mult)
            # fill rest with zeros
            v.memset(scrow[0:1, 11:NSC], 0.0)

            ps_sc = psum.tile([P, NSC], FP, tag="ps_sc")
            nc.tensor.matmul(ps_sc[:], ones_col[:], scrow[:], start=True, stop=True)
            g_sc = const.tile([P, NSC], FP, tag="g_sc")
            sc.copy(g_sc[:], ps_sc[:])

            MU0N = g_sc[:, 0:1]
            NRDMO = g_sc[:, 1:2]
            G = g_sc[:, 2:3]
            BMU = g_sc[:, 3:4]
            BO = g_sc[:, 4:5]
            RDOB = g_sc[:, 5:6]
            MUU = g_sc[:, 6:7]
            BB = g_sc[:, 7:8]
            TMO = g_sc[:, 8:9]
            TMB = g_sc[:, 9:10]
            NB02 = g_sc[:, 10:11]

            # ---------------- per-row pipeline part 2: coefficients -------
            stt = v.scalar_tensor_tensor
            tfa = st(0)              # t = f * alpha
            v.tensor_mul(tfa[:], f_all[:], alpha_all[:])
            v0 = st(1)
            v.tensor_mul(v0[:], f_all[:], x0_all[:])
            stt(v0[:], tfa[:], MU0N, v0[:], OP.mult, OP.add)
            c1 = st(2)
            v.tensor_scalar(c1[:], v0[:], NRDMO, None, OP.mult)
            m1 = st(1)
            v.tensor_sub(m1[:], c1[:], tfa[:])
            a_c = st(3)
            v.tensor_scalar(a_c[:], f_all[:], G, None, OP.mult)
            m_c = st(4)
            v.tensor_scalar(m_c[:], m1[:], G, None, OP.mult)
            c_c = st(5)
            v.tensor_scalar(c_c[:], c1[:], G, None, OP.mult)
            bv = st(0)
            v.tensor_mul(bv[:], a_c[:], xb_arr[:])
            stt(bv[:], m_c[:], BMU, bv[:], OP.mult, OP.subtract)
            stt(bv[:], c_c[:], BO, bv[:], OP.mult, OP.add)
            e1 = st(6)
            v.tensor_scalar(e1[:], bv[:], RDOB, None, OP.mult)
            c2 = st(2)
            v.tensor_add(c2[:], c_c[:], e1[:])

            # vnsq via Gram quadratic form (uses <x,x>_L = -1 on manifold)
            a2 = st(0)
            sc.activation(a2[:], a_c[:], AF.Square)
            m2 = st(1)
            sc.activation(m2[:], m_c[:], AF.Square)
            c2s = st(5)
            sc.activation(c2s[:], c2[:], AF.Square)
            b2s = st(7)
            sc.activation(b2s[:], e1[:], AF.Square)
            r = rt("r")
            v.tensor_scalar(r[:], a2[:], -1.0, None, OP.mult)
            stt(r[:], m2[:], MUU, r[:], OP.mult, OP.add)
            v.tensor_sub(r[:], r[:], c2s[:])
            stt(r[:], b2s[:], BB, r[:], OP.mult, OP.add)
            p = st(0)
            v.tensor_mul(p[:], a_c[:], m_c[:])
            v.tensor_mul(p[:], p[:], alpha_all[:])
            stt(r[:], p[:], -2.0, r[:], OP.mult, OP.add)
            v.tensor_mul(p[:], a_c[:], c2[:])
            v.tensor_mul(p[:], p[:], x0_all[:])
            stt(r[:], p[:], -2.0, r[:], OP.mult, OP.add)
            v.tensor_mul(p[:], a_c[:], e1[:])
            v.tensor_mul(p[:], p[:], xb_arr[:])
            stt(r[:], p[:], -2.0, r[:], OP.mult, OP.add)
            v.tensor_mul(p[:], m_c[:], c2[:])
            stt(r[:], p[:], TMO, r[:], OP.mult, OP.add)
            v.tensor_mul(p[:], m_c[:], e1[:])
            stt(r[:], p[:], TMB, r[:], OP.mult, OP.add)
            v.tensor_mul(p[:], c2[:], e1[:])
            stt(r[:], p[:], NB02, r[:], OP.mult, OP.add)
            v.tensor_scalar(r[:], r[:], 1e-9, None, OP.max)

            rvq = st(1)
            rvn = st(5)
            v.reciprocal(rvq[:], r[:])
            sc.activation(rvn[:], rvq[:], AF.Sqrt)
            nr_rsqrt(rvn, r)
            vn = st(0)
            v.tensor_mul(vn[:], r[:], rvn[:])

            E = st(7)
            sc.activation(E[:], vn[:], AF.Exp)
            Ei = st(1)
            sc.activation(Ei[:], vn[:], AF.Exp, scale=-1.0)
            v.tensor_scalar(E[:], E[:], 0.5, None, OP.mult)
            v.tensor_scalar(Ei[:], Ei[:], 0.5, None, OP.mult)
            Ch = st(0)
            v.tensor_add(Ch[:], E[:], Ei[:])
            Sh = st(8)
            v.tensor_sub(Sh[:], E[:], Ei[:])
            S = st(1)
            v.tensor_mul(S[:], Sh[:], rvn[:])

            A_all = rt("A_all")
            v.tensor_mul(A_all[:], S[:], a_c[:])
            B_all = rt("B_all")
            v.tensor_mul(B_all[:], S[:], m_c[:])
            D_all = st(7)
            v.tensor_mul(D_all[:], S[:], e1[:])
            v.tensor_add(D_all[:], D_all[:], Ch[:])
            if beta_origin:
                CD_all = rt("CD_all")
                v.tensor_mul(CD_all[:], S[:], c2[:])
                v.tensor_add(CD_all[:], CD_all[:], D_all[:])
            else:
                CD_all = rt("CD_all")  # beta coefficient D
                v.tensor_copy(CD_all[:], D_all[:])
                Cc_all = rt("Cc_all")  # e0 coefficient
                v.tensor_mul(Cc_all[:], S[:], c2[:])

            # ---------------- pass C: combine + write ---------------------
            mu_bc = mu_t[:].unsqueeze(1).broadcast_to([P, K, C])
            if not beta_origin:
                beta_bc = beta_t[:].unsqueeze(1).broadcast_to([P, K, C])
            for t in range(T):
                x_t = xpool.tile([P, F], FP, tag="x_t")
                nc.sync.dma_start(out=x_t[:], in_=x_view[t])
                x3 = x_t[:].rearrange("p (k c) -> p k c", c=C)
                out_t = opool.tile([P, F], FP, tag="out_t")
                o3 = out_t[:].rearrange("p (k c) -> p k c", c=C)
                A_bc = (
                    A_all[:, bass.ts(t, K)].unsqueeze(2).broadcast_to([P, K, C])
                )
                B_bc = (
                    B_all[:, bass.ts(t, K)].unsqueeze(2).broadcast_to([P, K, C])
                )
                v.tensor_mul(o3, x3, A_bc)
                t2 = wpool.tile([P, F], FP, tag="t2")
                t23 = t2[:].rearrange("p (k c) -> p k c", c=C)
                v.tensor_mul(t23, mu_bc, B_bc)
                v.tensor_add(o3, o3, t23)
                if beta_origin:
                    cd = CD_all[:, bass.ts(t, K)].unsqueeze(2)
                    v.tensor_add(o3[:, :, 0:1], o3[:, :, 0:1], cd)
                else:
                    D_bc = (
                        CD_all[:, bass.ts(t, K)]
                        .unsqueeze(2)
                        .broadcast_to([P, K, C])
                    )
                    v.tensor_mul(t23, beta_bc, D_bc)
                    v.tensor_add(o3, o3, t23)
                    cc_ = Cc_all[:, bass.ts(t, K)].unsqueeze(2)
                    v.tensor_add(o3[:, :, 0:1], o3[:, :, 0:1], cc_)
                nc.sync.dma_start(out=o_view[t], in_=out_t[:])

    nc.compile()
    return nc


_PROGRAM_CACHE: dict = {}


def _get_program(rows_per_core: int, n_total: int, beta_origin: bool) -> bass.Bass:
    key = (rows_per_core, n_total, beta_origin)
    if key not in _PROGRAM_CACHE:
        _PROGRAM_CACHE[key] = build_program(rows_per_core, n_total, beta_origin)
    return _PROGRAM_CACHE[key]


def run(x, gamma, beta, trace=False):
    x = np.ascontiguousarray(x, dtype=np.float32)
    gamma = np.ascontiguousarray(gamma, dtype=np.float32).reshape(1)
    beta = np.ascontiguousarray(beta, dtype=np.float32).reshape(-1)
    n, c = x.shape
    assert c == C and n % N_CORES == 0
    rpc = n // N_CORES
    beta_origin = bool(beta[0] == 1.0 and not np.any(beta[1:]))
    nc = _get_program(rpc, n, beta_origin)
    shards = x.reshape(N_CORES, rpc, c)
    in_maps = [
        {"x": shards[i], "gamma": gamma, "beta": beta} for i in range(N_CORES)
    ]
    res = run_bass_kernel_spmd(nc, in_maps, list(range(N_CORES)), trace=trace)
    out = np.concatenate([res.results[i]["out"] for i in range(N_CORES)], axis=0)
    return out, res


def kernel(x, gamma, beta):
    out, _ = run(x, gamma, beta)
    return out


# revision 17
# speedup vs baseline: 1.1835x; 1.1835x over previous
"""Trainium2 Bass kernel for Lorentz batch-norm (hyperboloid model).

Data-parallel over N across 8 NeuronCores. Per core:
  pass A: stream x, column-sum -> AllReduce -> Lorentz centroid mu
  pass B: stream x, per-row alpha = rowdot(x, J mu), d = arccosh(alpha),
          partial sum(d^2) -> AllReduce -> var -> div
  per-row coefficient pipeline: the whole transform collapses to
          out_row = A*x_row + B*mu + C*e0 + D*beta  (per-row scalars A..D)
  pass C: stream x, combine with broadcast APs, write out.

kernel(**inputs) takes FULL inputs (numpy), shards rows 8 ways, runs the
SPMD program via run_bass_kernel_spmd, concatenates the per-core outputs.
"""

import os
import sys

import numpy as np

for _p in ("/opt/trn_rl_repo", "/root/.axon_site/_ro/trn_rl_repo"):
    if os.path.isdir(_p) and _p not in sys.path:
        sys.path.insert(0, _p)

import concourse.bass as bass
import concourse.bacc as bacc
import concourse.tile as tile
from concourse import mybir
from concourse.bass_utils import run_bass_kernel_spmd

FP = mybir.dt.float32
N_CORES = 8
N_TOTAL = 1048576
C = 64
P = 128                      # SBUF partitions
K = 32                       # rows per partition per tile
EPS = 1e-5
ACOSH_EPS = 1e-7

AX = mybir.AxisListType
OP = mybir.AluOpType
AF = mybir.ActivationFunctionType


def build_program(rows_per_core: int, n_total: int, beta_origin: bool) -> bass.Bass:
    R = rows_per_core
    assert R % (P * K) == 0
    T = R // (P * K)          # number of tiles
    F = K * C                 # tile free size
    RPP = R // P              # per-row array width (= K*T)

    nc = bacc.Bacc(num_devices=N_CORES)

    x_in = nc.declare_dram_parameter("x", [R, C], FP, isOutput=False)
    g_in = nc.declare_dram_parameter("gamma", [1], FP, isOutput=False)
    b_in = nc.declare_dram_parameter("beta", [C], FP, isOutput=False)
    out_d = nc.declare_dram_parameter("out", [R, C], FP, isOutput=True)

    x_view = x_in.ap().rearrange("(t p k) c -> t p (k c)", p=P, k=K)
    o_view = out_d.ap().rearrange("(t p k) c -> t p (k c)", p=P, k=K)

    with tile.TileContext(nc) as tc:
        from contextlib import ExitStack

        with ExitStack() as ctx:
            const = ctx.enter_context(tc.tile_pool(name="const", bufs=1))
            xpool = ctx.enter_context(tc.tile_pool(name="xp", bufs=5))
            opool = ctx.enter_context(tc.tile_pool(name="op", bufs=3))
            wpool = ctx.enter_context(tc.tile_pool(name="wp", bufs=2))
            rows = ctx.enter_context(tc.tile_pool(name="rows", bufs=1))
            tmp = ctx.enter_context(tc.tile_pool(name="tmp", bufs=1))
            psum = ctx.enter_context(
                tc.tile_pool(name="ps", bufs=1, space="PSUM")
            )
            dram = ctx.enter_context(tc.tile_pool(name="dr", bufs=1, space="DRAM"))

            v = nc.vector
            sc = nc.scalar

            # ---------------- constants -----------------------------------
            ones_col = const.tile([1, P], FP, tag="ones_col")   # K=1 bcast lhsT
            v.memset(ones_col[:], 1.0)
            ones128 = const.tile([P, 1], FP, tag="ones128")     # col-sum lhsT
            v.memset(ones128[:], 1.0)
            jsign = const.tile([1, C], FP, tag="jsign")         # (1,-1,...,-1)
            v.memset(jsign[:], -1.0)
            v.memset(jsign[0:1, 0:1], 1.0)

            beta_row = const.tile([1, C], FP, tag="beta_row")
            nc.gpsimd.dma_start(out=beta_row[:], in_=b_in.ap().unsqueeze(0))
            gamma_sb = const.tile([1, 1], FP, tag="gamma_sb")
            nc.gpsimd.dma_start(out=gamma_sb[:], in_=g_in.ap().unsqueeze(0))
            jbeta_row = const.tile([1, C], FP, tag="jbeta_row")
            v.tensor_mul(jbeta_row[:], beta_row[:], jsign[:])

            # warm up the collective stream: first CC pays ~100us one-time
            # setup; run a zero-valued AllReduce that overlaps pass A, and
            # fold its (exactly 0.0) result into s_row to keep it live.
            wu_sb = tmp.tile([1, 1], FP, tag="wu_sb")
            v.memset(wu_sb[:], 0.0)
            wu_in = dram.tile([1, 1], FP, tag="wu_in")
            wu_out = dram.tile([1, 1], FP, tag="wu_out")
            nc.gpsimd.dma_start(out=wu_in[:], in_=wu_sb[:])
            nc.gpsimd.collective_compute(
                "AllReduce",
                OP.add,
                replica_groups=[list(range(N_CORES))],
                ins=[wu_in.opt()],
                outs=[wu_out.opt()],
            )
            wu_back = tmp.tile([1, 1], FP, tag="wu_back")
            nc.gpsimd.dma_start(out=wu_back[:], in_=wu_out[:])

            # ---------------- pass A: column sums on the PE ---------------
            NCHUNK = F // 512
            ps_a = psum.tile([1, F], FP, tag="ps_a")
            for t in range(T):
                x_t = xpool.tile([P, F], FP, tag="x_t")
                nc.sync.dma_start(out=x_t[:], in_=x_view[t])
                for q in range(NCHUNK):
                    nc.tensor.matmul(
                        ps_a[0:1, q * 512:(q + 1) * 512],
                        ones128[:],
                        x_t[:, q * 512:(q + 1) * 512],
                        start=(t == 0),
                        stop=(t == T - 1),
                    )
            sA = tmp.tile([1, F], FP, tag="sA")
            sc.copy(sA[:], ps_a[:])
            sA_t = sA[:].rearrange("p (k c) -> p k c", c=C).transpose([0, 2, 1])
            s_row = const.tile([1, C], FP, tag="s_row")
            v.tensor_reduce(s_row[:], sA_t, axis=AX.X, op=OP.add)
            # chain warmup result (adds exact 0.0)
            v.tensor_scalar(s_row[:], s_row[:], wu_back[0:1, 0:1], None, OP.add)

            # AllReduce s
            cc_in = dram.tile([1, C], FP, tag="cc_in")
            cc_out = dram.tile([1, C], FP, tag="cc_out")
            nc.gpsimd.dma_start(out=cc_in[:], in_=s_row[:])
            nc.gpsimd.collective_compute(
                "AllReduce",
                OP.add,
                replica_groups=[list(range(N_CORES))],
                ins=[cc_in.opt()],
                outs=[cc_out.opt()],
            )
            sg_row = const.tile([1, C], FP, tag="sg_row")
            nc.gpsimd.dma_start(out=sg_row[:], in_=cc_out[:])

            # ---------------- mu on partition 0 ---------------------------
            def nr_rsqrt(y, z, steps=2):
                """Refine y ~= 1/sqrt(z) in place (per-element tiles)."""
                t1 = tmp.tile(list(y.shape), FP, tag="nr_t1", name="nr_t1")
                for _ in range(steps):
                    v.tensor_mul(t1[:], y[:], y[:])
                    v.tensor_mul(t1[:], t1[:], z[:])
                    v.tensor_scalar(t1[:], t1[:], -0.5, 1.5, OP.mult, OP.add)
                    v.tensor_mul(y[:], y[:], t1[:])

            sq64 = tmp.tile([1, C], FP, tag="sq64")
            sc.activation(sq64[:], sg_row[:], AF.Square)
            ssum = tmp.tile([1, 1], FP, tag="ssum")
            v.tensor_reduce(ssum[:], sq64[:], axis=AX.X, op=OP.add)
            snsq = tmp.tile([1, 1], FP, tag="snsq")
            # snsq = -<s,s>_L = 2*s0^2 - sum(s^2)
            v.tensor_scalar(snsq[:], sq64[0:1, 0:1], 2.0, None, OP.mult)
            v.tensor_sub(snsq[:], snsq[:], ssum[:])
            v.tensor_scalar(snsq[:], snsq[:], 1e-9, None, OP.max)
            rsn = tmp.tile([1, 1], FP, tag="rsn")
            rtmp = tmp.tile([1, 1], FP, tag="rtmp")
            v.reciprocal(rtmp[:], snsq[:])
            sc.activation(rsn[:], rtmp[:], AF.Sqrt)
            nr_rsqrt(rsn, snsq)

            mu_row = const.tile([1, C], FP, tag="mu_row")
            v.tensor_scalar(mu_row[:], sg_row[:], rsn[0:1, 0:1], None, OP.mult)
            jmu_row = const.tile([1, C], FP, tag="jmu_row")
            v.tensor_mul(jmu_row[:], mu_row[:], jsign[:])

            # broadcast mu, Jmu to all partitions via PE
            ps_b = psum.tile([P, C], FP, tag="ps_b")
            mu_t = const.tile([P, C], FP, tag="mu_t")
            nc.tensor.matmul(ps_b[:], ones_col[:], mu_row[:], start=True, stop=True)
            sc.copy(mu_t[:], ps_b[:])
            ps_b2 = psum.tile([P, C], FP, tag="ps_b2")
            jmu_t = const.tile([P, C], FP, tag="jmu_t")
            nc.tensor.matmul(ps_b2[:], ones_col[:], jmu_row[:], start=True, stop=True)
            sc.copy(jmu_t[:], ps_b2[:])

            if not beta_origin:
                jbeta_t = const.tile([P, C], FP, tag="jbeta_t")
                ps_b3 = psum.tile([P, C], FP, tag="ps_b3")
                nc.tensor.matmul(
                    ps_b3[:], ones_col[:], jbeta_row[:], start=True, stop=True
                )
                sc.copy(jbeta_t[:], ps_b3[:])
                beta_t = const.tile([P, C], FP, tag="beta_t")
                ps_b4 = psum.tile([P, C], FP, tag="ps_b4")
                nc.tensor.matmul(
                    ps_b4[:], ones_col[:], beta_row[:], start=True, stop=True
                )
                sc.copy(beta_t[:], ps_b4[:])

            # ---------------- pass B: alpha, x0 (and xb) ------------------
            alpha_all = rows.tile([P, RPP], FP, tag="alpha_all")
            x0_all = rows.tile([P, RPP], FP, tag="x0_all")
            if not beta_origin:
                xb_all = rows.tile([P, RPP], FP, tag="xb_all")
            jmu_bc = jmu_t[:].unsqueeze(1).broadcast_to([P, K, C])
            if not beta_origin:
                jbeta_bc = jbeta_t[:].unsqueeze(1).broadcast_to([P, K, C])

            for t in range(T):
                x_t = xpool.tile([P, F], FP, tag="x_t")
                nc.sync.dma_start(out=x_t[:], in_=x_view[t])
                x3 = x_t[:].rearrange("p (k c) -> p k c", c=C)
                xm = wpool.tile([P, F], FP, tag="xm", bufs=3)
                xm3 = xm[:].rearrange("p (k c) -> p k c", c=C)
                # split the mul between DVE and GpSimd to balance engines
                if t % 2 == 0:
                    nc.gpsimd.tensor_tensor(xm3, x3, jmu_bc, OP.mult)
                else:
                    v.tensor_mul(xm3, x3, jmu_bc)
                v.tensor_reduce(
                    alpha_all[:, bass.ts(t, K)], xm3, axis=AX.X, op=OP.add
                )
                v.tensor_copy(x0_all[:, bass.ts(t, K)], x3[:, :, 0:1].squeeze(2))
                if not beta_origin:
                    xm2 = wpool.tile([P, F], FP, tag="xm2")
                    xm23 = xm2[:].rearrange("p (k c) -> p k c", c=C)
                    nc.gpsimd.tensor_tensor(xm23, x3, jbeta_bc, OP.mult)
                    v.tensor_reduce(
                        xb_all[:, bass.ts(t, K)], xm23, axis=AX.X, op=OP.add
                    )

            xb_arr = x0_all if beta_origin else xb_all

            # ---------------- per-row pipeline part 1: d, f, var ----------
            def rt(tag):
                return rows.tile([P, RPP], FP, tag=tag, name=tag)

            # scratch slots, reused aggressively (bufs=1, WAR deps serialize)
            def st(i):
                return rows.tile([P, RPP], FP, tag=f"s{i}", name=f"s{i}")

            z = st(0)
            # clip alpha
            v.tensor_scalar(alpha_all[:], alpha_all[:], 1.0 + ACOSH_EPS, None, OP.max)
            sc.activation(z[:], alpha_all[:], AF.Square)
            v.tensor_scalar(z[:], z[:], -1.0, ACOSH_EPS, OP.add, OP.max)
            rz = st(1)
            rsq = st(2)
            v.reciprocal(rz[:], z[:])
            sc.activation(rsq[:], rz[:], AF.Sqrt)
            nr_rsqrt(rsq, z, steps=1)
            sqz = st(1)
            v.tensor_mul(sqz[:], z[:], rsq[:])
            w = st(3)
            v.tensor_add(w[:], alpha_all[:], sqz[:])
            d = st(0)
            sc.activation(d[:], w[:], AF.Ln)
            f_all = rt("f_all")
            v.tensor_mul(f_all[:], d[:], rsq[:])

            vacc = tmp.tile([P, 1], FP, tag="vacc")
            dsq_scr = st(1)
            v.tensor_mul(dsq_scr[:], d[:], d[:])
            v.tensor_reduce(vacc[:], dsq_scr[:], axis=AX.X, op=OP.add)
            ps_v = psum.tile([1, 1], FP, tag="ps_v")
            nc.tensor.matmul(ps_v[:], ones128[:], vacc[:], start=True, stop=True)
            vs_row = tmp.tile([1, 1], FP, tag="vs_row")
            sc.copy(vs_row[:], ps_v[:])

            vcc_in = dram.tile([1, 1], FP, tag="vcc_in")
            vcc_out = dram.tile([1, 1], FP, tag="vcc_out")
            nc.gpsimd.dma_start(out=vcc_in[:], in_=vs_row[:])
            nc.gpsimd.collective_compute(
                "AllReduce",
                OP.add,
                replica_groups=[list(range(N_CORES))],
                ins=[vcc_in.opt()],
                outs=[vcc_out.opt()],
            )
            vsum = tmp.tile([1, 1], FP, tag="vsum")
            nc.gpsimd.dma_start(out=vsum[:], in_=vcc_out[:])

            # div = sqrt(var + EPS); G = gamma / div
            dv = tmp.tile([1, 1], FP, tag="dv")
            v.tensor_scalar(dv[:], vsum[:], 1.0 / float(n_total), EPS, OP.mult, OP.add)
            rdd = tmp.tile([1, 1], FP, tag="rdd")
            v.reciprocal(rdd[:], dv[:])
            rdiv = tmp.tile([1, 1], FP, tag="rdiv")
            sc.activation(rdiv[:], rdd[:], AF.Sqrt)
            nr_rsqrt(rdiv, dv)

            # ---------------- global scalar row + broadcast ---------------
            NSC = 16
            scrow = const.tile([1, NSC], FP, tag="scrow")
            mu0 = mu_row[0:1, 0:1]
            b0 = beta_row[0:1, 0:1]
            # 0: -mu0
            v.tensor_scalar(scrow[0:1, 0:1], mu0, -1.0, None, OP.mult)
            # 1: -1/(1+mu0)
            t11 = tmp.tile([1, 1], FP, tag="t11")
            v.tensor_scalar(t11[:], mu0, 1.0, None, OP.add)
            t12 = tmp.tile([1, 1], FP, tag="t12")
            v.reciprocal(t12[:], t11[:])
            v.tensor_scalar(scrow[0:1, 1:2], t12[:], -1.0, None, OP.mult)
            # 2: G = gamma * rdiv
            v.tensor_mul(scrow[0:1, 2:3], gamma_sb[:], rdiv[:])
            # 3: bmu = <beta,mu>_L = -sum(beta*Jmu)
            t64 = tmp.tile([1, C], FP, tag="t64")
            v.tensor_mul(t64[:], beta_row[:], jmu_row[:])
            t13 = tmp.tile([1, 1], FP, tag="t13")
            v.tensor_reduce(t13[:], t64[:], axis=AX.X, op=OP.add)
            v.tensor_scalar(scrow[0:1, 3:4], t13[:], -1.0, None, OP.mult)
            # 4: bo = -beta0
            v.tensor_scalar(scrow[0:1, 4:5], b0, -1.0, None, OP.mult)
            # 5: rdob = 1/(1+beta0)
            v.tensor_scalar(t11[:], b0, 1.0, None, OP.add)
            v.reciprocal(scrow[0:1, 5:6], t11[:])
            # 6: muu = <mu,mu>_L = -sum(mu*Jmu)
            v.tensor_mul(t64[:], mu_row[:], jmu_row[:])
            v.tensor_reduce(t13[:], t64[:], axis=AX.X, op=OP.add)
            v.tensor_scalar(scrow[0:1, 6:7], t13[:], -1.0, None, OP.mult)
            # 7: bb = <beta,beta>_L = -sum(beta*Jbeta)
            v.tensor_mul(t64[:], beta_row[:], jbeta_row[:])
            v.tensor_reduce(t13[:], t64[:], axis=AX.X, op=OP.add)
            v.tensor_scalar(scrow[0:1, 7:8], t13[:], -1.0, None, OP.mult)
            # 8: tmo = -2*mu0
            v.tensor_scalar(scrow[0:1, 8:9], mu0, -2.0, None, OP.mult)
            # 9: tmb = 2*bmu
            v.tensor_scalar(scrow[0:1, 9:10], scrow[0:1, 3:4], 2.0, None, OP.mult)
            # 10: -2*beta0
            v.tensor_scalar(scrow[0:1, 10:11], b0, -2.0, None, OP.mult)
            # fill rest with zeros
            v.memset(scrow[0:1, 11:NSC], 0.0)

            ps_sc = psum.tile([P, NSC], FP, tag="ps_sc")
            nc.tensor.matmul(ps_sc[:], ones_col[:], scrow[:], start=True, stop=True)
            g_sc = const.tile([P, NSC], FP, tag="g_sc")
            sc.copy(g_sc[:], ps_sc[:])

            MU0N = g_sc[:, 0:1]
            NRDMO = g_sc[:, 1:2]
            G = g_sc[:, 2:3]
            BMU = g_sc[:, 3:4]
            BO = g_sc[:, 4:5]
            RDOB = g_sc[:, 5:6]
            MUU = g_sc[:, 6:7]
            BB = g_sc[:, 7:8]
            TMO = g_sc[:, 8:9]
            TMB = g_sc[:, 9:10]
            NB02 = g_sc[:, 10:11]

            # ---------------- per-row pipeline part 2: coefficients -------
            stt = v.scalar_tensor_tensor
            tfa = st(0)              # t = f * alpha
            v.tensor_mul(tfa[:], f_all[:], alpha_all[:])
            v0 = st(1)
            v.tensor_mul(v0[:], f_all[:], x0_all[:])
            stt(v0[:], tfa[:], MU0N, v0[:], OP.mult, OP.add)
            c1 = st(2)
            v.tensor_scalar(c1[:], v0[:], NRDMO, None, OP.mult)
            m1 = st(1)
            v.tensor_sub(m1[:], c1[:], tfa[:])
            a_c = st(3)
            v.tensor_scalar(a_c[:], f_all[:], G, None, OP.mult)
            m_c = st(4)
            v.tensor_scalar(m_c[:], m1[:], G, None, OP.mult)
            c_c = st(5)
            v.tensor_scalar(c_c[:], c1[:], G, None, OP.mult)
            bv = st(0)
            v.tensor_mul(bv[:], a_c[:], xb_arr[:])
            stt(bv[:], m_c[:], BMU, bv[:], OP.mult, OP.subtract)
            stt(bv[:], c_c[:], BO, bv[:], OP.mult, OP.add)
            e1 = st(6)
            v.tensor_scalar(e1[:], bv[:], RDOB, None, OP.mult)
            c2 = st(2)
            v.tensor_add(c2[:], c_c[:], e1[:])

            # vnsq via Gram quadratic form (uses <x,x>_L = -1 on manifold)
            a2 = st(0)
            sc.activation(a2[:], a_c[:], AF.Square)
            m2 = st(1)
            sc.activation(m2[:], m_c[:], AF.Square)
            c2s = st(5)
            sc.activation(c2s[:], c2[:], AF.Square)
            b2s = st(7)
            sc.activation(b2s[:], e1[:], AF.Square)
            r = rt("r")
            v.tensor_scalar(r[:], a2[:], -1.0, None, OP.mult)
            stt(r[:], m2[:], MUU, r[:], OP.mult, OP.add)
            v.tensor_sub(r[:], r[:], c2s[:])
            stt(r[:], b2s[:], BB, r[:], OP.mult, OP.add)
            p = st(0)
            v.tensor_mul(p[:], a_c[:], m_c[:])
            v.tensor_mul(p[:], p[:], alpha_all[:])
            stt(r[:], p[:], -2.0, r[:], OP.mult, OP.add)
            v.tensor_mul(p[:], a_c[:], c2[:])
            v.tensor_mul(p[:], p[:], x0_all[:])
            stt(r[:], p[:], -2.0, r[:], OP.mult, OP.add)
            v.tensor_mul(p[:], a_c[:], e1[:])
            v.tensor_mul(p[:], p[:], xb_arr[:])
            stt(r[:], p[:], -2.0, r[:], OP.mult, OP.add)
            v.tensor_mul(p[:], m_c[:], c2[:])
            stt(r[:], p[:], TMO, r[:], OP.mult, OP.add)
            v.tensor_mul(p[:], m_c[:], e1[:])
            stt(r[:], p[:], TMB, r[:], OP.mult, OP.add)
            v.tensor_mul(p[:], c2[:], e1[:])
            stt(r[:], p[:], NB02, r[:], OP.mult, OP.add)
            v.tensor_scalar(r[:], r[:], 1e-9, None, OP.max)

            rvq = st(1)
            rvn = st(5)
            v.reciprocal(rvq[:], r[:])
            sc.activation(rvn[:], rvq[:], AF.Sqrt)
            nr_rsqrt(rvn, r, steps=1)
            vn = st(0)
            v.tensor_mul(vn[:], r[:], rvn[:])

            E = st(7)
            sc.activation(E[:], vn[:], AF.Exp)
            Ei = st(1)
            sc.activation(Ei[:], vn[:], AF.Exp, scale=-1.0)
            v.tensor_scalar(E[:], E[:], 0.5, None, OP.mult)
            v.tensor_scalar(Ei[:], Ei[:], 0.5, None, OP.mult)
            Ch = st(0)
            v.tensor_add(Ch[:], E[:], Ei[:])
            Sh = st(8)
            v.tensor_sub(Sh[:], E[:], Ei[:])
            S = st(1)
            v.tensor_mul(S[:], Sh[:], rvn[:])

            A_all = rt("A_all")
            v.tensor_mul(A_all[:], S[:], a_c[:])
            B_all = rt("B_all")
            v.tensor_mul(B_all[:], S[:], m_c[:])
            D_all = st(7)
            v.tensor_mul(D_all[:], S[:], e1[:])
            v.tensor_add(D_all[:], D_all[:], Ch[:])
            if beta_origin:
                CD_all = rt("CD_all")
                v.tensor_mul(CD_all[:], S[:], c2[:])
                v.tensor_add(CD_all[:], CD_all[:], D_all[:])
            else:
                CD_all = rt("CD_all")  # beta coefficient D
                v.tensor_copy(CD_all[:], D_all[:])
                Cc_all = rt("Cc_all")  # e0 coefficient
                v.tensor_mul(Cc_all[:], S[:], c2[:])

            # ---------------- pass C: combine + write ---------------------
            mu_bc = mu_t[:].unsqueeze(1).broadcast_to([P, K, C])
            if not beta_origin:
                beta_bc = beta_t[:].unsqueeze(1).broadcast_to([P, K, C])
            for t in range(T):
                x_t = xpool.tile([P, F], FP, tag="x_t")
                nc.sync.dma_start(out=x_t[:], in_=x_view[t])
                x3 = x_t[:].rearrange("p (k c) -> p k c", c=C)
                out_t = opool.tile([P, F], FP, tag="out_t")
                o3 = out_t[:].rearrange("p (k c) -> p k c", c=C)
                A_bc = (
                    A_all[:, bass.ts(t, K)].unsqueeze(2).broadcast_to([P, K, C])
                )
                B_bc = (
                    B_all[:, bass.ts(t, K)].unsqueeze(2).broadcast_to([P, K, C])
                )
                v.tensor_mul(o3, x3, A_bc)
                t2 = wpool.tile([P, F], FP, tag="t2")
                t23 = t2[:].rearrange("p (k c) -> p k c", c=C)
                nc.gpsimd.tensor_tensor(t23, mu_bc, B_bc, OP.mult)
                v.tensor_add(o3, o3, t23)
                if beta_origin:
                    cd = CD_all[:, bass.ts(t, K)].unsqueeze(2)
                    v.tensor_add(o3[:, :, 0:1], o3[:, :, 0:1], cd)
                else:
                    D_bc = (
                        CD_all[:, bass.ts(t, K)]
                        .unsqueeze(2)
                        .broadcast_to([P, K, C])
                    )
                    v.tensor_mul(t23, beta_bc, D_bc)
                    v.tensor_add(o3, o3, t23)
                    cc_ = Cc_all[:, bass.ts(t, K)].unsqueeze(2)
                    v.tensor_add(o3[:, :, 0:1], o3[:, :, 0:1], cc_)
                nc.sync.dma_start(out=o_view[t], in_=out_t[:])

    nc.compile()
    return nc


_PROGRAM_CACHE: dict = {}


def _get_program(rows_per_core: int, n_total: int, beta_origin: bool) -> bass.Bass:
    key = (rows_per_core, n_total, beta_origin)
    if key not in _PROGRAM_CACHE:
        _PROGRAM_CACHE[key] = build_program(rows_per_core, n_total, beta_origin)
    return _PROGRAM_CACHE[key]


def run(x, gamma, beta, trace=False):
    x = np.ascontiguousarray(x, dtype=np.float32)
    gamma = np.ascontiguousarray(gamma, dtype=np.float32).reshape(1)
    beta = np.ascontiguousarray(beta, dtype=np.float32).reshape(-1)
    n, c = x.shape
    assert c == C and n % N_CORES == 0
    rpc = n // N_CORES
    beta_origin = bool(beta[0] == 1.0 and not np.any(beta[1:]))
    nc = _get_program(rpc, n, beta_origin)
    shards = x.reshape(N_CORES, rpc, c)
    in_maps = [
        {"x": shards[i], "gamma": gamma, "beta": beta} for i in range(N_CORES)
    ]
    res = run_bass_kernel_spmd(nc, in_maps, list(range(N_CORES)), trace=trace)
    out = np.concatenate([res.results[i]["out"] for i in range(N_CORES)], axis=0)
    return out, res


def kernel(x, gamma, beta):
    out, _ = run(x, gamma, beta)
    return out


# revision 21
# speedup vs baseline: 1.2648x; 1.0687x over previous
"""Trainium2 Bass kernel for Lorentz batch-norm (hyperboloid model).

Data-parallel over N across 8 NeuronCores. Per core:
  pass A: stream x, column-sum -> AllReduce -> Lorentz centroid mu
  pass B: stream x, per-row alpha = rowdot(x, J mu), d = arccosh(alpha),
          partial sum(d^2) -> AllReduce -> var -> div
  per-row coefficient pipeline: the whole transform collapses to
          out_row = A*x_row + B*mu + C*e0 + D*beta  (per-row scalars A..D)
  pass C: stream x, combine with broadcast APs, write out.

kernel(**inputs) takes FULL inputs (numpy), shards rows 8 ways, runs the
SPMD program via run_bass_kernel_spmd, concatenates the per-core outputs.
"""

import os
import sys

import numpy as np

for _p in ("/opt/trn_rl_repo", "/root/.axon_site/_ro/trn_rl_repo"):
    if os.path.isdir(_p) and _p not in sys.path:
        sys.path.insert(0, _p)

import concourse.bass as bass
import concourse.bacc as bacc
import concourse.tile as tile
from concourse import mybir
from concourse.bass_utils import run_bass_kernel_spmd

FP = mybir.dt.float32
N_CORES = 8
N_TOTAL = 1048576
C = 64
P = 128                      # SBUF partitions
K = 32                       # rows per partition per tile
EPS = 1e-5
ACOSH_EPS = 1e-7

AX = mybir.AxisListType
OP = mybir.AluOpType
AF = mybir.ActivationFunctionType


def build_program(rows_per_core: int, n_total: int, beta_origin: bool) -> bass.Bass:
    R = rows_per_core
    assert R % (P * K) == 0
    T = R // (P * K)          # number of tiles
    F = K * C                 # tile free size
    RPP = R // P              # per-row array width (= K*T)

    nc = bacc.Bacc(num_devices=N_CORES)

    x_in = nc.declare_dram_parameter("x", [R, C], FP, isOutput=False)
    g_in = nc.declare_dram_parameter("gamma", [1], FP, isOutput=False)
    b_in = nc.declare_dram_parameter("beta", [C], FP, isOutput=False)
    out_d = nc.declare_dram_parameter("out", [R, C], FP, isOutput=True)

    x_view = x_in.ap().rearrange("(t p k) c -> t p (k c)", p=P, k=K)
    o_view = out_d.ap().rearrange("(t p k) c -> t p (k c)", p=P, k=K)

    with tile.TileContext(nc) as tc:
        from contextlib import ExitStack

        with ExitStack() as ctx:
            const = ctx.enter_context(tc.tile_pool(name="const", bufs=1))
            xpool = ctx.enter_context(tc.tile_pool(name="xp", bufs=6))
            opool = ctx.enter_context(tc.tile_pool(name="op", bufs=3))
            wpool = ctx.enter_context(tc.tile_pool(name="wp", bufs=2))
            rows = ctx.enter_context(tc.tile_pool(name="rows", bufs=1))
            tmp = ctx.enter_context(tc.tile_pool(name="tmp", bufs=1))
            psum = ctx.enter_context(
                tc.tile_pool(name="ps", bufs=1, space="PSUM")
            )
            dram = ctx.enter_context(tc.tile_pool(name="dr", bufs=1, space="DRAM"))

            v = nc.vector
            sc = nc.scalar

            # ---------------- constants -----------------------------------
            ones_col = const.tile([1, P], FP, tag="ones_col")   # K=1 bcast lhsT
            v.memset(ones_col[:], 1.0)
            ones128 = const.tile([P, 1], FP, tag="ones128")     # col-sum lhsT
            v.memset(ones128[:], 1.0)
            jsign = const.tile([1, C], FP, tag="jsign")         # (1,-1,...,-1)
            v.memset(jsign[:], -1.0)
            v.memset(jsign[0:1, 0:1], 1.0)

            beta_row = const.tile([1, C], FP, tag="beta_row")
            nc.gpsimd.dma_start(out=beta_row[:], in_=b_in.ap().unsqueeze(0))
            gamma_sb = const.tile([1, 1], FP, tag="gamma_sb")
            nc.gpsimd.dma_start(out=gamma_sb[:], in_=g_in.ap().unsqueeze(0))
            jbeta_row = const.tile([1, C], FP, tag="jbeta_row")
            v.tensor_mul(jbeta_row[:], beta_row[:], jsign[:])

            # warm up the collective stream: first CC pays ~100us one-time
            # setup; run a zero-valued AllReduce that overlaps pass A, and
            # fold its (exactly 0.0) result into s_row to keep it live.
            wu_sb = tmp.tile([1, 1], FP, tag="wu_sb")
            v.memset(wu_sb[:], 0.0)
            wu_in = dram.tile([1, 1], FP, tag="wu_in")
            wu_out = dram.tile([1, 1], FP, tag="wu_out")
            nc.gpsimd.dma_start(out=wu_in[:], in_=wu_sb[:])
            nc.gpsimd.collective_compute(
                "AllReduce",
                OP.add,
                replica_groups=[list(range(N_CORES))],
                ins=[wu_in.opt()],
                outs=[wu_out.opt()],
            )
            wu_back = tmp.tile([1, 1], FP, tag="wu_back")
            nc.gpsimd.dma_start(out=wu_back[:], in_=wu_out[:])
            # second warmup so the first *real* collective runs at steady cost
            wu2_in = dram.tile([1, 1], FP, tag="wu2_in")
            wu2_out = dram.tile([1, 1], FP, tag="wu2_out")
            nc.gpsimd.dma_start(out=wu2_in[:], in_=wu_back[:])
            nc.gpsimd.collective_compute(
                "AllReduce",
                OP.add,
                replica_groups=[list(range(N_CORES))],
                ins=[wu2_in.opt()],
                outs=[wu2_out.opt()],
            )
            wu2_back = tmp.tile([1, 1], FP, tag="wu2_back")
            nc.gpsimd.dma_start(out=wu2_back[:], in_=wu2_out[:])

            # ---------------- pass A: column sums (PE + DVE split) --------
            HF = F // 2
            ps_a = psum.tile([1, HF], FP, tag="ps_a")
            acc = rows.tile([P, HF], FP, tag="acc")
            for t in range(T):
                x_t = xpool.tile([P, F], FP, tag="x_t")
                nc.sync.dma_start(out=x_t[:], in_=x_view[t])
                for q in range(HF // 512):
                    nc.tensor.matmul(
                        ps_a[0:1, q * 512:(q + 1) * 512],
                        ones128[:],
                        x_t[:, q * 512:(q + 1) * 512],
                        start=(t == 0),
                        stop=(t == T - 1),
                    )
                if t == 0:
                    v.tensor_copy(acc[:], x_t[:, HF:F])
                else:
                    v.tensor_add(acc[:], acc[:], x_t[:, HF:F])
            # fold PE half: [1, HF] -> [1, C]
            sA = tmp.tile([1, HF], FP, tag="sA")
            sc.copy(sA[:], ps_a[:])
            sA_t = sA[:].rearrange("p (k c) -> p k c", c=C).transpose([0, 2, 1])
            s_row = const.tile([1, C], FP, tag="s_row")
            v.tensor_reduce(s_row[:], sA_t, axis=AX.X, op=OP.add)
            # fold DVE half: [P, HF] -> [P, C] -> matmul ones -> [1, C]
            acc_t = acc[:].rearrange("p (k c) -> p k c", c=C).transpose([0, 2, 1])
            s_p = tmp.tile([P, C], FP, tag="s_p")
            v.tensor_reduce(s_p[:], acc_t, axis=AX.X, op=OP.add)
            ps_s = psum.tile([1, C], FP, tag="ps_s")
            nc.tensor.matmul(ps_s[:], ones128[:], s_p[:], start=True, stop=True)
            s_row2 = tmp.tile([1, C], FP, tag="s_row2")
            sc.copy(s_row2[:], ps_s[:])
            v.tensor_add(s_row[:], s_row[:], s_row2[:])
            # chain warmup result (adds exact 0.0)
            v.tensor_scalar(s_row[:], s_row[:], wu2_back[0:1, 0:1], None, OP.add)

            # AllReduce s
            cc_in = dram.tile([1, C], FP, tag="cc_in")
            cc_out = dram.tile([1, C], FP, tag="cc_out")
            nc.gpsimd.dma_start(out=cc_in[:], in_=s_row[:])
            nc.gpsimd.collective_compute(
                "AllReduce",
                OP.add,
                replica_groups=[list(range(N_CORES))],
                ins=[cc_in.opt()],
                outs=[cc_out.opt()],
            )
            sg_row = const.tile([1, C], FP, tag="sg_row")
            nc.gpsimd.dma_start(out=sg_row[:], in_=cc_out[:])

            # ---------------- mu on partition 0 ---------------------------
            def nr_rsqrt(y, z, steps=2):
                """Refine y ~= 1/sqrt(z) in place (per-element tiles)."""
                t1 = tmp.tile(list(y.shape), FP, tag="nr_t1", name="nr_t1")
                for _ in range(steps):
                    v.tensor_mul(t1[:], y[:], y[:])
                    v.tensor_mul(t1[:], t1[:], z[:])
                    v.tensor_scalar(t1[:], t1[:], -0.5, 1.5, OP.mult, OP.add)
                    v.tensor_mul(y[:], y[:], t1[:])

            sq64 = tmp.tile([1, C], FP, tag="sq64")
            sc.activation(sq64[:], sg_row[:], AF.Square)
            ssum = tmp.tile([1, 1], FP, tag="ssum")
            v.tensor_reduce(ssum[:], sq64[:], axis=AX.X, op=OP.add)
            snsq = tmp.tile([1, 1], FP, tag="snsq")
            # snsq = -<s,s>_L = 2*s0^2 - sum(s^2)
            v.tensor_scalar(snsq[:], sq64[0:1, 0:1], 2.0, None, OP.mult)
            v.tensor_sub(snsq[:], snsq[:], ssum[:])
            v.tensor_scalar(snsq[:], snsq[:], 1e-9, None, OP.max)
            rsn = tmp.tile([1, 1], FP, tag="rsn")
            rtmp = tmp.tile([1, 1], FP, tag="rtmp")
            v.reciprocal(rtmp[:], snsq[:])
            sc.activation(rsn[:], rtmp[:], AF.Sqrt)
            nr_rsqrt(rsn, snsq)

            mu_row = const.tile([1, C], FP, tag="mu_row")
            v.tensor_scalar(mu_row[:], sg_row[:], rsn[0:1, 0:1], None, OP.mult)
            jmu_row = const.tile([1, C], FP, tag="jmu_row")
            v.tensor_mul(jmu_row[:], mu_row[:], jsign[:])

            # broadcast mu, Jmu to all partitions via PE
            ps_b = psum.tile([P, C], FP, tag="ps_b")
            mu_t = const.tile([P, C], FP, tag="mu_t")
            nc.tensor.matmul(ps_b[:], ones_col[:], mu_row[:], start=True, stop=True)
            sc.copy(mu_t[:], ps_b[:])
            ps_b2 = psum.tile([P, C], FP, tag="ps_b2")
            jmu_t = const.tile([P, C], FP, tag="jmu_t")
            nc.tensor.matmul(ps_b2[:], ones_col[:], jmu_row[:], start=True, stop=True)
            sc.copy(jmu_t[:], ps_b2[:])

            if not beta_origin:
                jbeta_t = const.tile([P, C], FP, tag="jbeta_t")
                ps_b3 = psum.tile([P, C], FP, tag="ps_b3")
                nc.tensor.matmul(
                    ps_b3[:], ones_col[:], jbeta_row[:], start=True, stop=True
                )
                sc.copy(jbeta_t[:], ps_b3[:])
                beta_t = const.tile([P, C], FP, tag="beta_t")
                ps_b4 = psum.tile([P, C], FP, tag="ps_b4")
                nc.tensor.matmul(
                    ps_b4[:], ones_col[:], beta_row[:], start=True, stop=True
                )
                sc.copy(beta_t[:], ps_b4[:])

            # ---------------- pass B: alpha, x0 (and xb) ------------------
            alpha_all = rows.tile([P, RPP], FP, tag="alpha_all")
            x0_all = rows.tile([P, RPP], FP, tag="x0_all")
            if not beta_origin:
                xb_all = rows.tile([P, RPP], FP, tag="xb_all")
            jmu_bc = jmu_t[:].unsqueeze(1).broadcast_to([P, K, C])
            if not beta_origin:
                jbeta_bc = jbeta_t[:].unsqueeze(1).broadcast_to([P, K, C])

            for t in range(T):
                x_t = xpool.tile([P, F], FP, tag="x_t")
                nc.sync.dma_start(out=x_t[:], in_=x_view[t])
                x3 = x_t[:].rearrange("p (k c) -> p k c", c=C)
                xm = wpool.tile([P, F], FP, tag="xm", bufs=3)
                xm3 = xm[:].rearrange("p (k c) -> p k c", c=C)
                # split the mul between DVE and GpSimd to balance engines
                if t % 2 == 0:
                    nc.gpsimd.tensor_tensor(xm3, x3, jmu_bc, OP.mult)
                else:
                    v.tensor_mul(xm3, x3, jmu_bc)
                v.tensor_reduce(
                    alpha_all[:, bass.ts(t, K)], xm3, axis=AX.X, op=OP.add
                )
                sc.copy(x0_all[:, bass.ts(t, K)], x3[:, :, 0:1].squeeze(2))
                if not beta_origin:
                    xm2 = wpool.tile([P, F], FP, tag="xm2")
                    xm23 = xm2[:].rearrange("p (k c) -> p k c", c=C)
                    nc.gpsimd.tensor_tensor(xm23, x3, jbeta_bc, OP.mult)
                    v.tensor_reduce(
                        xb_all[:, bass.ts(t, K)], xm23, axis=AX.X, op=OP.add
                    )

            xb_arr = x0_all if beta_origin else xb_all

            # ---------------- per-row pipeline part 1: d, f, var ----------
            def rt(tag):
                return rows.tile([P, RPP], FP, tag=tag, name=tag)

            # scratch slots, reused aggressively (bufs=1, WAR deps serialize)
            def st(i):
                return rows.tile([P, RPP], FP, tag=f"s{i}", name=f"s{i}")

            z = st(0)
            # clip alpha
            v.tensor_scalar(alpha_all[:], alpha_all[:], 1.0 + ACOSH_EPS, None, OP.max)
            sc.activation(z[:], alpha_all[:], AF.Square)
            v.tensor_scalar(z[:], z[:], -1.0, ACOSH_EPS, OP.add, OP.max)
            rz = st(1)
            rsq = st(2)
            v.reciprocal(rz[:], z[:])
            sc.activation(rsq[:], rz[:], AF.Sqrt)
            nr_rsqrt(rsq, z, steps=1)
            sqz = st(1)
            v.tensor_mul(sqz[:], z[:], rsq[:])
            w = st(3)
            v.tensor_add(w[:], alpha_all[:], sqz[:])
            d = st(0)
            sc.activation(d[:], w[:], AF.Ln)
            f_all = rt("f_all")
            v.tensor_mul(f_all[:], d[:], rsq[:])

            vacc = tmp.tile([P, 1], FP, tag="vacc")
            dsq_scr = st(1)
            v.tensor_mul(dsq_scr[:], d[:], d[:])
            v.tensor_reduce(vacc[:], dsq_scr[:], axis=AX.X, op=OP.add)
            ps_v = psum.tile([1, 1], FP, tag="ps_v")
            nc.tensor.matmul(ps_v[:], ones128[:], vacc[:], start=True, stop=True)
            vs_row = tmp.tile([1, 1], FP, tag="vs_row")
            sc.copy(vs_row[:], ps_v[:])

            vcc_in = dram.tile([1, 1], FP, tag="vcc_in")
            vcc_out = dram.tile([1, 1], FP, tag="vcc_out")
            nc.gpsimd.dma_start(out=vcc_in[:], in_=vs_row[:])
            nc.gpsimd.collective_compute(
                "AllReduce",
                OP.add,
                replica_groups=[list(range(N_CORES))],
                ins=[vcc_in.opt()],
                outs=[vcc_out.opt()],
            )
            vsum = tmp.tile([1, 1], FP, tag="vsum")
            nc.gpsimd.dma_start(out=vsum[:], in_=vcc_out[:])

            # div = sqrt(var + EPS); G = gamma / div
            dv = tmp.tile([1, 1], FP, tag="dv")
            v.tensor_scalar(dv[:], vsum[:], 1.0 / float(n_total), EPS, OP.mult, OP.add)
            rdd = tmp.tile([1, 1], FP, tag="rdd")
            v.reciprocal(rdd[:], dv[:])
            rdiv = tmp.tile([1, 1], FP, tag="rdiv")
            sc.activation(rdiv[:], rdd[:], AF.Sqrt)
            nr_rsqrt(rdiv, dv)

            # ---------------- global scalar row + broadcast ---------------
            NSC = 16
            scrow = const.tile([1, NSC], FP, tag="scrow")
            mu0 = mu_row[0:1, 0:1]
            b0 = beta_row[0:1, 0:1]
            # 0: -mu0
            v.tensor_scalar(scrow[0:1, 0:1], mu0, -1.0, None, OP.mult)
            # 1: -1/(1+mu0)
            t11 = tmp.tile([1, 1], FP, tag="t11")
            v.tensor_scalar(t11[:], mu0, 1.0, None, OP.add)
            t12 = tmp.tile([1, 1], FP, tag="t12")
            v.reciprocal(t12[:], t11[:])
            v.tensor_scalar(scrow[0:1, 1:2], t12[:], -1.0, None, OP.mult)
            # 2: G = gamma * rdiv
            v.tensor_mul(scrow[0:1, 2:3], gamma_sb[:], rdiv[:])
            # 3: bmu = <beta,mu>_L = -sum(beta*Jmu)
            t64 = tmp.tile([1, C], FP, tag="t64")
            v.tensor_mul(t64[:], beta_row[:], jmu_row[:])
            t13 = tmp.tile([1, 1], FP, tag="t13")
            v.tensor_reduce(t13[:], t64[:], axis=AX.X, op=OP.add)
            v.tensor_scalar(scrow[0:1, 3:4], t13[:], -1.0, None, OP.mult)
            # 4: bo = -beta0
            v.tensor_scalar(scrow[0:1, 4:5], b0, -1.0, None, OP.mult)
            # 5: rdob = 1/(1+beta0)
            v.tensor_scalar(t11[:], b0, 1.0, None, OP.add)
            v.reciprocal(scrow[0:1, 5:6], t11[:])
            # 6: muu = <mu,mu>_L = -sum(mu*Jmu)
            v.tensor_mul(t64[:], mu_row[:], jmu_row[:])
            v.tensor_reduce(t13[:], t64[:], axis=AX.X, op=OP.add)
            v.tensor_scalar(scrow[0:1, 6:7], t13[:], -1.0, None, OP.mult)
            # 7: bb = <beta,beta>_L = -sum(beta*Jbeta)
            v.tensor_mul(t64[:], beta_row[:], jbeta_row[:])
            v.tensor_reduce(t13[:], t64[:], axis=AX.X, op=OP.add)
            v.tensor_scalar(scrow[0:1, 7:8], t13[:], -1.0, None, OP.mult)
            # 8: tmo = -2*mu0
            v.tensor_scalar(scrow[0:1, 8:9], mu0, -2.0, None, OP.mult)
            # 9: tmb = 2*bmu
            v.tensor_scalar(scrow[0:1, 9:10], scrow[0:1, 3:4], 2.0, None, OP.mult)
            # 10: -2*beta0
            v.tensor_scalar(scrow[0:1, 10:11], b0, -2.0, None, OP.mult)
            # fill rest with zeros
            v.memset(scrow[0:1, 11:NSC], 0.0)

            ps_sc = psum.tile([P, NSC], FP, tag="ps_sc")
            nc.tensor.matmul(ps_sc[:], ones_col[:], scrow[:], start=True, stop=True)
            g_sc = const.tile([P, NSC], FP, tag="g_sc")
            sc.copy(g_sc[:], ps_sc[:])

            MU0N = g_sc[:, 0:1]
            NRDMO = g_sc[:, 1:2]
            G = g_sc[:, 2:3]
            BMU = g_sc[:, 3:4]
            BO = g_sc[:, 4:5]
            RDOB = g_sc[:, 5:6]
            MUU = g_sc[:, 6:7]
            BB = g_sc[:, 7:8]
            TMO = g_sc[:, 8:9]
            TMB = g_sc[:, 9:10]
            NB02 = g_sc[:, 10:11]

            # ---------------- per-row pipeline part 2: coefficients -------
            stt = v.scalar_tensor_tensor
            tfa = st(0)              # t = f * alpha
            v.tensor_mul(tfa[:], f_all[:], alpha_all[:])
            v0 = st(1)
            v.tensor_mul(v0[:], f_all[:], x0_all[:])
            stt(v0[:], tfa[:], MU0N, v0[:], OP.mult, OP.add)
            c1 = st(2)
            v.tensor_scalar(c1[:], v0[:], NRDMO, None, OP.mult)
            m1 = st(1)
            v.tensor_sub(m1[:], c1[:], tfa[:])
            a_c = st(3)
            v.tensor_scalar(a_c[:], f_all[:], G, None, OP.mult)
            m_c = st(4)
            v.tensor_scalar(m_c[:], m1[:], G, None, OP.mult)
            c_c = st(5)
            v.tensor_scalar(c_c[:], c1[:], G, None, OP.mult)
            bv = st(0)
            v.tensor_mul(bv[:], a_c[:], xb_arr[:])
            stt(bv[:], m_c[:], BMU, bv[:], OP.mult, OP.subtract)
            stt(bv[:], c_c[:], BO, bv[:], OP.mult, OP.add)
            e1 = st(6)
            v.tensor_scalar(e1[:], bv[:], RDOB, None, OP.mult)
            c2 = st(2)
            v.tensor_add(c2[:], c_c[:], e1[:])

            # vnsq via Gram quadratic form (uses <x,x>_L = -1 on manifold)
            a2 = st(0)
            sc.activation(a2[:], a_c[:], AF.Square)
            m2 = st(1)
            sc.activation(m2[:], m_c[:], AF.Square)
            c2s = st(5)
            sc.activation(c2s[:], c2[:], AF.Square)
            b2s = st(7)
            sc.activation(b2s[:], e1[:], AF.Square)
            r = rt("r")
            v.tensor_scalar(r[:], a2[:], -1.0, None, OP.mult)
            stt(r[:], m2[:], MUU, r[:], OP.mult, OP.add)
            v.tensor_sub(r[:], r[:], c2s[:])
            stt(r[:], b2s[:], BB, r[:], OP.mult, OP.add)
            p = st(0)
            v.tensor_mul(p[:], a_c[:], m_c[:])
            v.tensor_mul(p[:], p[:], alpha_all[:])
            stt(r[:], p[:], -2.0, r[:], OP.mult, OP.add)
            v.tensor_mul(p[:], a_c[:], c2[:])
            v.tensor_mul(p[:], p[:], x0_all[:])
            stt(r[:], p[:], -2.0, r[:], OP.mult, OP.add)
            v.tensor_mul(p[:], a_c[:], e1[:])
            v.tensor_mul(p[:], p[:], xb_arr[:])
            stt(r[:], p[:], -2.0, r[:], OP.mult, OP.add)
            v.tensor_mul(p[:], m_c[:], c2[:])
            stt(r[:], p[:], TMO, r[:], OP.mult, OP.add)
            v.tensor_mul(p[:], m_c[:], e1[:])
            stt(r[:], p[:], TMB, r[:], OP.mult, OP.add)
            v.tensor_mul(p[:], c2[:], e1[:])
            stt(r[:], p[:], NB02, r[:], OP.mult, OP.add)
            v.tensor_scalar(r[:], r[:], 1e-9, None, OP.max)

            rvq = st(1)
            rvn = st(5)
            v.reciprocal(rvq[:], r[:])
            sc.activation(rvn[:], rvq[:], AF.Sqrt)
            nr_rsqrt(rvn, r, steps=1)
            vn = st(0)
            v.tensor_mul(vn[:], r[:], rvn[:])

            E = st(7)
            sc.activation(E[:], vn[:], AF.Exp)
            Ei = st(1)
            sc.activation(Ei[:], vn[:], AF.Exp, scale=-1.0)
            v.tensor_scalar(E[:], E[:], 0.5, None, OP.mult)
            v.tensor_scalar(Ei[:], Ei[:], 0.5, None, OP.mult)
            Ch = st(0)
            v.tensor_add(Ch[:], E[:], Ei[:])
            Sh = st(8)
            v.tensor_sub(Sh[:], E[:], Ei[:])
            S = st(1)
            v.tensor_mul(S[:], Sh[:], rvn[:])

            A_all = rt("A_all")
            v.tensor_mul(A_all[:], S[:], a_c[:])
            B_all = rt("B_all")
            v.tensor_mul(B_all[:], S[:], m_c[:])
            D_all = st(7)
            v.tensor_mul(D_all[:], S[:], e1[:])
            v.tensor_add(D_all[:], D_all[:], Ch[:])
            if beta_origin:
                CD_all = rt("CD_all")
                v.tensor_mul(CD_all[:], S[:], c2[:])
                v.tensor_add(CD_all[:], CD_all[:], D_all[:])
            else:
                CD_all = rt("CD_all")  # beta coefficient D
                v.tensor_copy(CD_all[:], D_all[:])
                Cc_all = rt("Cc_all")  # e0 coefficient
                v.tensor_mul(Cc_all[:], S[:], c2[:])

            # ---------------- pass C: combine + write ---------------------
            mu_bc = mu_t[:].unsqueeze(1).broadcast_to([P, K, C])
            if not beta_origin:
                beta_bc = beta_t[:].unsqueeze(1).broadcast_to([P, K, C])
            for t in range(T):
                x_t = xpool.tile([P, F], FP, tag="x_t")
                nc.sync.dma_start(out=x_t[:], in_=x_view[t])
                x3 = x_t[:].rearrange("p (k c) -> p k c", c=C)
                out_t = opool.tile([P, F], FP, tag="out_t")
                o3 = out_t[:].rearrange("p (k c) -> p k c", c=C)
                A_bc = (
                    A_all[:, bass.ts(t, K)].unsqueeze(2).broadcast_to([P, K, C])
                )
                B_bc = (
                    B_all[:, bass.ts(t, K)].unsqueeze(2).broadcast_to([P, K, C])
                )
                v.tensor_mul(o3, x3, A_bc)
                t2 = wpool.tile([P, F], FP, tag="t2")
                t23 = t2[:].rearrange("p (k c) -> p k c", c=C)
                v.tensor_mul(t23, mu_bc, B_bc)
                v.tensor_add(o3, o3, t23)
                if beta_origin:
                    cd = CD_all[:, bass.ts(t, K)].unsqueeze(2)
                    v.tensor_add(o3[:, :, 0:1], o3[:, :, 0:1], cd)
                else:
                    D_bc = (
                        CD_all[:, bass.ts(t, K)]
                        .unsqueeze(2)
                        .broadcast_to([P, K, C])
                    )
                    v.tensor_mul(t23, beta_bc, D_bc)
                    v.tensor_add(o3, o3, t23)
                    cc_ = Cc_all[:, bass.ts(t, K)].unsqueeze(2)
                    v.tensor_add(o3[:, :, 0:1], o3[:, :, 0:1], cc_)
                nc.sync.dma_start(out=o_view[t], in_=out_t[:])

    nc.compile()
    return nc


_PROGRAM_CACHE: dict = {}


def _get_program(rows_per_core: int, n_total: int, beta_origin: bool) -> bass.Bass:
    key = (rows_per_core, n_total, beta_origin)
    if key not in _PROGRAM_CACHE:
        _PROGRAM_CACHE[key] = build_program(rows_per_core, n_total, beta_origin)
    return _PROGRAM_CACHE[key]


def run(x, gamma, beta, trace=False):
    x = np.ascontiguousarray(x, dtype=np.float32)
    gamma = np.ascontiguousarray(gamma, dtype=np.float32).reshape(1)
    beta = np.ascontiguousarray(beta, dtype=np.float32).reshape(-1)
    n, c = x.shape
    assert c == C and n % N_CORES == 0
    rpc = n // N_CORES
    beta_origin = bool(beta[0] == 1.0 and not np.any(beta[1:]))
    nc = _get_program(rpc, n, beta_origin)
    shards = x.reshape(N_CORES, rpc, c)
    in_maps = [
        {"x": shards[i], "gamma": gamma, "beta": beta} for i in range(N_CORES)
    ]
    res = run_bass_kernel_spmd(nc, in_maps, list(range(N_CORES)), trace=trace)
    out = np.concatenate([res.results[i]["out"] for i in range(N_CORES)], axis=0)
    return out, res


def kernel(x, gamma, beta):
    out, _ = run(x, gamma, beta)
    return out
